# revision 13
# baseline (speedup 1.0000x reference)
"""Trainium2 Bass kernel for an image-captioning LSTM decoder (v2).

Model (per reference):
  emb = embedding[captions]                      [B, T, E]
  sum_enc = encoder_out.sum(axis=1)              [B, ENC]
  h0 = mean_enc @ W_h0.T + b_h0 ; c0 likewise
  per step t (Tdec = T-1 steps):
    gates = [emb_t, sum_enc] @ W_ih.T + b_ih + h @ W_hh.T + b_hh
    i,f,g,o -> LSTM update; rows with t >= caption_len-1 give preds 0
    preds_t = h_new @ W_fc.T + b_fc  (masked)

Sharding: data-parallel over batch; core c owns rows {c, c+8, ...}.
Weights replicated; no inter-core communication.

v2 design (vs v1):
  * bf16 matmul operands everywhere (gates/fc/EG); fp32 psum accum and
    fp32 pointwise state.  Numpy-simulated rel err ~2.3e-3 (gate 2e-2).
  * "L layout" [32, 128] tiles for the pointwise chain: partition
    p = 8j + r holds (row r, feature-chunk j); DVE/ACT ops are ~2.5x
    cheaper than on [8, 512] tiles (measured 326 vs 805 ns).
  * Gate preacts computed straight into one [32, 512] PSUM tile
    (cols = [g|i|f|o]): first an identity-matmul injects EG_t (the
    precomputed emb/enc/bias part, fp32r), then 64 bf16 matmuls of
    128 cols accumulate h @ W_hh.T on top.  No DVE pre-adds; ACT reads
    PSUM directly.
  * One PE transpose per step ([32,128] -> [128,32]) instead of four,
    one cast into HTall (the bf16 hidden-state history that feeds both
    the next step's stationaries and the fc GEMM).
  * Host precomputes sum_enc / h0 / c0 / xenc (phase A eliminated);
    per-step EG_L tiles are gathered from pos-major EG via 4 small
    SBUF->SBUF DMAs, prefetched 2 steps ahead.
  * Phase B (EG GEMM) is split into 16 one-bank pieces; piece (0, *)
    runs up front, the rest fill the PE tail of steps 0..15 the same
    way fc chunks fill steps 16..62.
  * Masking moved out of the recurrence: inactive rows run free (their
    h is never read), preds are zeroed by folding the position mask
    into the fc PSUM->SBUF copy (tensor_scalar_mul / ACT-copy scale).
"""

import numpy as np
import ml_dtypes
from contextlib import ExitStack

import concourse.bass as bass
import concourse.tile as tile
from concourse import mybir, bacc
from concourse.bass_utils import run_bass_kernel_spmd
from concourse.masks import make_identity

F32 = mybir.dt.float32
F32R = mybir.dt.float32r
BF16 = mybir.dt.bfloat16
BF = ml_dtypes.bfloat16

NCORES = 8
B, T, V, E, D, ENC, P = 64, 64, 10000, 512, 512, 512, 196
TD = T - 1            # 63 decode steps
R = B // NCORES       # 8 rows per core
POS = TD * R          # 504 positions per core
KD = D // 128         # 4 k-chunks of the hidden dim
VCH = 500             # vocab chunk (psum bank holds 512 fp32)
NV = V // VCH         # 20 chunks
MTS = [128, 128, 128, POS - 384]   # fc position tiles (504 = 3*128 + 120)

_PROG_CACHE = {}


def _build_program(with_bfc: bool):
    nc = bacc.Bacc("TRN2", target_bir_lowering=False, debug=False,
                   num_devices=NCORES)

    def inp(name, shape, dt=F32):
        return nc.dram_tensor(name, shape, dt, kind="ExternalInput").ap()

    embT = inp("embT", [KD, 128, POS], BF16)
    wembT = inp("wembT", [KD, 128, 4 * D], BF16)
    whhT = inp("whhT", [KD, 128, 4 * D], BF16)
    wfcT = inp("wfcT", [KD, 128, V], BF16)
    xenc16 = inp("xenc16", [128, 4 * D], F32R)
    h0T = inp("h0T", [128, KD, R], BF16)
    c0L = inp("c0L", [128, 128], F32)
    sel32 = inp("sel32", [32, 128], F32R)
    maskP = inp("maskP", [128, 4], F32)
    if with_bfc:
        bfc = inp("bfc", [1, V], BF16)
        m1 = inp("m1", [1, POS], BF16)
    preds = nc.dram_tensor("preds", [R, TD, V], F32, kind="ExternalOutput").ap()
    preds_trv = preds.rearrange("r t v -> t r v")

    SIG = mybir.ActivationFunctionType.Sigmoid
    TANH = mybir.ActivationFunctionType.Tanh
    COPY = mybir.ActivationFunctionType.Copy

    with tile.TileContext(nc) as tc, ExitStack() as ctx:
        const_pool = ctx.enter_context(tc.tile_pool(name="const", bufs=1))
        state_pool = ctx.enter_context(tc.tile_pool(name="state", bufs=1))
        egl_pool = ctx.enter_context(tc.tile_pool(name="egl", bufs=4))
        pw_pool = ctx.enter_context(tc.tile_pool(name="pw", bufs=2))
        phd_out = ctx.enter_context(tc.tile_pool(name="phd_out", bufs=3))
        gps_pool = ctx.enter_context(tc.tile_pool(name="gps", bufs=2, space="PSUM"))
        tps_pool = ctx.enter_context(tc.tile_pool(name="tps", bufs=1, space="PSUM"))
        fps_pool = ctx.enter_context(tc.tile_pool(name="fps", bufs=3, space="PSUM"))
        bps_pool = ctx.enter_context(tc.tile_pool(name="bps", bufs=1, space="PSUM"))

        ENGS = [nc.sync, nc.gpsimd, nc.scalar, nc.sync]

        # ---- constants / state ----
        ident128 = const_pool.tile([128, 128], F32, name="ident128")
        make_identity(nc, ident128[:])
        identr128 = const_pool.tile([128, 128], F32R, name="identr128")
        nc.vector.tensor_copy(identr128[:], ident128[:])
        sel_sb = const_pool.tile([32, 128], F32R, name="sel_sb")
        nc.sync.dma_start(sel_sb[:], sel32[:])
        maskP_sb = const_pool.tile([128, 4], F32, name="maskP_sb")
        nc.sync.dma_start(maskP_sb[:], maskP[:])
        xenc_sb = const_pool.tile([128, 4 * D], F32R, name="xenc_sb")
        nc.gpsimd.dma_start(xenc_sb[:], xenc16[:])

        # HTall[:, k, 8s+r] = (h before step s).T chunk k, bf16 (k-major
        # so matmul stationary slices are 1-D contiguous)
        HTall = state_pool.tile([128, KD, T * R], BF16, name="HTall")
        nc.scalar.dma_start(HTall[:, :, 0:R], h0T[:])
        c_sb = state_pool.tile([128, 128], F32, name="c_sb")
        nc.gpsimd.dma_start(c_sb[:], c0L[:])

        whh_sb = [state_pool.tile([128, 4 * D], BF16, name=f"whh{k}")
                  for k in range(KD)]
        for k in range(KD):
            ENGS[k].dma_start(whh_sb[k][:], whhT[k])

        # resident bf16 W_fc: 80KB/partition, loaded once in n-block order
        wfc_sb = [state_pool.tile([128, V], BF16, name=f"wfc{k}")
                  for k in range(KD)]
        for blk in range(4):
            for k in range(KD):
                ENGS[(blk + k) % 3].dma_start(
                    wfc_sb[k][:, 2500 * blk:2500 * (blk + 1)],
                    wfcT[k][:, 2500 * blk:2500 * (blk + 1)])

        # EG pos-major tiles: EG[m][8*s + r, 512j + 128n + c]
        EG = [state_pool.tile([128, 4 * D], F32R, name=f"EG{m}")
              for m in range(4)]
        # phase-B operands, loaded piecewise (m- / sl-granular)
        emb_sb = [state_pool.tile([128, POS], BF16, name=f"emb{k}")
                  for k in range(KD)]
        wemb_sb = [state_pool.tile([128, 4 * D], BF16, name=f"wemb{k}")
                   for k in range(KD)]

        def load_emb_piece(m):
            p0, pn = 128 * m, MTS[m]
            for k in range(KD):
                ENGS[k].dma_start(emb_sb[k][:, p0:p0 + pn],
                                  embT[k][:, p0:p0 + pn])

        def load_wemb_piece(sl):
            c0_, c1 = 512 * sl, 512 * (sl + 1)
            for k in range(KD):
                ENGS[(k + 2) % 4].dma_start(wemb_sb[k][:, c0_:c1],
                                            wembT[k][:, c0_:c1])

        def eg_piece_mm(m, sl):
            mw = MTS[m]
            ps = bps_pool.tile([128, 512], F32, name=f"egp{m}_{sl}", tag="egp")
            for k in range(KD):
                nc.tensor.matmul(ps[:mw, :],
                                 emb_sb[k][:, 128 * m:128 * m + mw],
                                 wemb_sb[k][:, 512 * sl:512 * (sl + 1)],
                                 start=(k == 0), stop=(k == KD - 1))
            return (ps, m, sl)

        def eg_piece_add(job):
            ps, m, sl = job
            mw = MTS[m]
            nc.vector.tensor_add(EG[m][:mw, 512 * sl:512 * (sl + 1)],
                                 ps[:mw, :], xenc_sb[:mw, 512 * sl:512 * (sl + 1)])

        def eg_piece(m, sl):
            eg_piece_add(eg_piece_mm(m, sl))

        if with_bfc:
            bfc_sb = const_pool.tile([1, V], BF16, name="bfc_sb")
            nc.sync.dma_start(bfc_sb[:], bfc[:])
            m1_sb = const_pool.tile([1, POS], BF16, name="m1_sb")
            nc.sync.dma_start(m1_sb[:], m1[:])

        def fc_mm(m, n):
            """fc matmuls for (pos-tile m, vocab chunk n); copy deferred."""
            mw = MTS[m]
            ps = fps_pool.tile([128, VCH], F32, name=f"fc{n}_{m}", tag="fcps")
            for k in range(KD):
                nc.tensor.matmul(
                    ps[:mw, :],
                    HTall[:, k, R * (16 * m + 1):R * (16 * m + 1) + mw],
                    wfc_sb[k][:, VCH * n:VCH * (n + 1)], start=(k == 0),
                    stop=(k == KD - 1 and not with_bfc))
            if with_bfc:
                nc.tensor.matmul(
                    ps[:mw, :], m1_sb[:, 128 * m:128 * m + mw],
                    bfc_sb[:, VCH * n:VCH * (n + 1)],
                    start=False, stop=True)
            return (ps, m, n)

        def fc_finish(job):
            """Masked PSUM->SBUF copy + output DMA (run after the casts)."""
            ps, m, n = job
            mw = MTS[m]
            stn = mw // R
            ot = phd_out.tile([128, VCH], F32, name=f"fo{n}_{m}", tag="fcout")
            if n % 2 == 0:
                nc.vector.tensor_scalar_mul(ot[:mw, :], ps[:mw, :],
                                            maskP_sb[:mw, m:m + 1])
            else:
                nc.scalar.activation(ot[:mw, :], ps[:mw, :], COPY,
                                     scale=maskP_sb[:mw, m:m + 1])
            t0 = 16 * m
            nc.gpsimd.dma_start(
                preds_trv[t0:t0 + stn, :, VCH * n:VCH * (n + 1)],
                ot[:mw, :])

        def egl_prefetch(t):
            """Gather EG_L[t] [32, 512] (partition 8j+r) from pos-major EG."""
            egl = egl_pool.tile([32, 512], F32R, name=f"egl{t}", tag="egl")
            m, s = t // 16, t % 16
            for j in range(KD):
                nc.sync.dma_start(egl[8 * j:8 * (j + 1), :],
                                  EG[m][8 * s:8 * s + 8, 512 * j:512 * (j + 1)])
            return egl

        # ---- phase B piece (0, *) up front; EG_L prefetch for steps 0,1 ----
        load_emb_piece(0)
        load_wemb_piece(0)
        eg_piece(0, 0)
        load_wemb_piece(1)
        eg_piece(0, 1)
        load_wemb_piece(2)
        eg_piece(0, 2)
        load_wemb_piece(3)
        eg_piece(0, 3)
        egl_tiles = {0: egl_prefetch(0), 1: egl_prefetch(1)}
        for m in range(1, 4):
            load_emb_piece(m)

        def g_inject(t):
            """Start step t's gate PSUM: G = sel32.T @ EG_L[t].

            G [128, 512]: partition 32j + r (rows r>=8 of each quadrant are
            zero garbage), col 128n + c."""
            G = gps_pool.tile([128, 512], F32, name=f"G{t}", tag="G")
            nc.tensor.matmul(G[:, :], sel_sb[:], egl_tiles[t][:],
                             start=True, stop=False, skip_group_check=True)
            return G

        # remaining phase-B pieces scheduled into steps 0..11 (one per step);
        # fc chunks into steps 16..62 (tile m during steps 16(m+1)..)
        PIECES = [(m, sl) for m in range(1, 4) for sl in range(4)]

        G_next = g_inject(0)
        for t in range(TD):
            G = G_next

            # gates: G[8j:8j+8, 128n:...] += hT_k.T @ Whh slice, bf16
            for k in range(KD):
                lhs = HTall[:, k, R * t:R * t + R]
                last = (k == KD - 1)
                for n in range(4):
                    if last and n == 3:
                        break
                    for j in range(KD):
                        nc.tensor.matmul(
                            G[32 * j:32 * j + 8, 128 * n:128 * (n + 1)],
                            lhs, whh_sb[k][:, 512 * j + 128 * n:
                                           512 * j + 128 * n + 128],
                            start=False, stop=last, skip_group_check=True,
                            tile_position=(0, 32 * j))

            # ACT on g/i/f while the o-bank matmuls + fillers stream
            TG = pw_pool.tile([128, 128], F32, name=f"tg{t}", tag="tg")
            nc.scalar.activation(TG[:], G[:, 0:128], TANH)
            SIF = pw_pool.tile([128, 256], F32, name=f"sif{t}", tag="sif")
            nc.scalar.activation(SIF[:], G[:, 128:384], SIG)

            # o-bank (k=3, n=3)
            lhs = HTall[:, KD - 1, R * t:R * t + R]
            for j in range(KD):
                nc.tensor.matmul(
                    G[32 * j:32 * j + 8, 384:512],
                    lhs, whh_sb[KD - 1][:, 512 * j + 384:512 * j + 512],
                    start=False, stop=True, skip_group_check=True,
                    tile_position=(0, 32 * j))

            # c path on DVE; tanh(c) queued on ACT before sigmoid(o) so the
            # h-mul's inputs are both ready right after the o matmuls land
            T1 = pw_pool.tile([128, 128], F32, name=f"t1_{t}", tag="t1")
            nc.vector.tensor_mul(T1[:], TG[:], SIF[:, 0:128])
            T2 = pw_pool.tile([128, 128], F32, name=f"t2_{t}", tag="t2")
            nc.vector.tensor_mul(T2[:], SIF[:, 128:256], c_sb[:])
            nc.vector.tensor_add(c_sb[:], T1[:], T2[:])
            TC = pw_pool.tile([128, 128], F32, name=f"tc{t}", tag="tc")
            nc.scalar.activation(TC[:], c_sb[:], TANH)
            SO = pw_pool.tile([128, 128], F32, name=f"so{t}", tag="so")
            nc.scalar.activation(SO[:], G[:, 384:512], SIG)

            # PE fillers: phase-B pieces (steps 2..13) / fc chunks (16..).
            # One filler + the next inject run before the transpose (covering
            # the sig_o -> h window); extra fc chunks go after it.  The
            # PSUM->SBUF copies are deferred past the HTall casts so the DVE
            # queue stays clear for the h chain.
            deferred = []
            fillers = []
            if 2 <= t < 2 + len(PIECES):
                fillers.append(("piece", PIECES[t - 2]))
            m = t // 16 - 1
            if m >= 0:
                L = min(16 * (m + 2), TD) - 16 * (m + 1)
                s = t - 16 * (m + 1)
                for n in range(s * NV // L, (s + 1) * NV // L):
                    fillers.append(("fc", (m, n)))
            if fillers:
                kind, arg = fillers[0]
                if kind == "fc":
                    deferred.append(("fc", fc_mm(*arg)))
                else:
                    deferred.append(("piece", eg_piece_mm(*arg)))
            # next step's EG inject can run before this step's transpose
            if t + 1 < TD:
                G_next = g_inject(t + 1)

            HB = pw_pool.tile([128, 128], F32R, name=f"h{t}", tag="hb")
            nc.vector.tensor_mul(HB[:], TC[:], SO[:])

            tp = tps_pool.tile([128, 128], F32R, name=f"hT{t}", tag="htp")
            nc.tensor.transpose(tp[:, :], HB[:], identr128[:])
            for k in range(KD):
                nc.vector.tensor_copy(
                    HTall[:, k, R * (t + 1):R * (t + 2)],
                    tp[:, 32 * k:32 * k + R])

            for kind, arg in fillers[1:]:
                if kind == "fc":
                    deferred.append(("fc", fc_mm(*arg)))
                else:
                    deferred.append(("piece", eg_piece_mm(*arg)))
            for kind, job in deferred:
                if kind == "fc":
                    fc_finish(job)
                else:
                    eg_piece_add(job)

            if t + 2 < TD:
                egl_tiles[t + 2] = egl_prefetch(t + 2)
            egl_tiles.pop(t, None)

        for n in range(NV):
            fc_finish(fc_mm(3, n))

    nc.compile()
    return nc


GPERM = None  # row permutation [g, i, f, o] built lazily


def _gate_perm():
    global GPERM
    if GPERM is None:
        gp = np.concatenate([np.arange(2 * D, 3 * D), np.arange(0, D),
                             np.arange(D, 2 * D), np.arange(3 * D, 4 * D)])
        # [n, j, c] -> [j, n, c]
        GPERM = gp.reshape(4, 4, 128).transpose(1, 0, 2).reshape(4 * D)
    return GPERM


def _chunkT(w, dt=BF):
    """[N, K<=512] weight -> transposed chunks [KD, 128, N]."""
    wt = np.ascontiguousarray(w.T.astype(np.float32))
    return wt.reshape(KD, 128, w.shape[0]).astype(dt)


def kernel(encoder_out, encoder_captions, caption_len, embedding,
           W_ih, b_ih, W_hh, b_hh, W_h0, b_h0, W_c0, b_c0, W_fc, b_fc):
    encoder_out = np.asarray(encoder_out, dtype=np.float32)
    encoder_captions = np.asarray(encoder_captions)
    caption_len = np.asarray(caption_len)
    embedding = np.asarray(embedding, dtype=np.float32)
    W_ih = np.asarray(W_ih, dtype=np.float32); b_ih = np.asarray(b_ih, np.float32)
    W_hh = np.asarray(W_hh, dtype=np.float32); b_hh = np.asarray(b_hh, np.float32)
    W_h0 = np.asarray(W_h0, dtype=np.float32); b_h0 = np.asarray(b_h0, np.float32)
    W_c0 = np.asarray(W_c0, dtype=np.float32); b_c0 = np.asarray(b_c0, np.float32)
    W_fc = np.asarray(W_fc, dtype=np.float32); b_fc = np.asarray(b_fc, np.float32)

    with_bfc = bool(np.any(b_fc != 0))
    key = with_bfc
    if key not in _PROG_CACHE:
        _PROG_CACHE[key] = _build_program(with_bfc)
    nc = _PROG_CACHE[key]

    perm = _gate_perm()
    whhT = _chunkT(W_hh[perm])
    wembT = _chunkT(W_ih[perm, :E])
    wfcT = _chunkT(W_fc)

    # host phase A
    sum_enc = encoder_out.sum(axis=1)                       # [B, ENC]
    mean_enc = sum_enc / np.float32(P)
    h0 = mean_enc @ W_h0.T + b_h0                            # [B, D]
    c0 = mean_enc @ W_c0.T + b_c0
    xenc = (sum_enc @ W_ih[:, E:].T + b_ih + b_hh)[:, perm]  # [B, 4D]

    in_maps = []
    all_rows = []
    for c in range(NCORES):
        rows = np.arange(c, B, NCORES)
        all_rows.append(rows)
        cap = np.asarray(encoder_captions[rows][:, :TD], dtype=np.int64)
        embg = embedding[cap]                                # [R, TD, E]
        embT = np.ascontiguousarray(
            embg.transpose(2, 1, 0).reshape(E, POS)).reshape(
                KD, 128, POS).astype(BF)
        dec_len = (caption_len[rows] - 1).astype(np.int64)
        mp = np.zeros((64, R), np.float32)
        mp[:TD] = (np.arange(TD)[:, None] < dec_len[None, :])
        maskP = np.ascontiguousarray(
            mp.reshape(4, 16, R).transpose(1, 2, 0).reshape(128, 4))
        h0c = h0[rows]                                       # [R, D]
        h0T = np.ascontiguousarray(
            h0c.reshape(R, KD, 128).transpose(2, 1, 0)).astype(BF)  # [128,KD,R]
        c0c = c0[rows]
        c0L = np.zeros((KD, 32, 128), np.float32)
        c0L[:, :R, :] = c0c.reshape(R, KD, 128).transpose(1, 0, 2)
        c0L = np.ascontiguousarray(c0L.reshape(128, 128))
        xenc16 = np.ascontiguousarray(np.tile(xenc[rows], (16, 1)))  # [128,4D]
        sel32 = np.zeros((32, 128), np.float32)
        for j in range(KD):
            for r in range(R):
                sel32[8 * j + r, 32 * j + r] = 1.0
        im = dict(embT=embT, wembT=wembT, whhT=whhT, wfcT=wfcT,
                  xenc16=xenc16, h0T=h0T, c0L=c0L, maskP=maskP, sel32=sel32)
        if with_bfc:
            im["bfc"] = b_fc.reshape(1, V).astype(BF)
            im["m1"] = np.ones((1, POS), BF)
        in_maps.append(im)

    global _LAST_IN_MAPS
    _LAST_IN_MAPS = in_maps
    res = run_bass_kernel_spmd(nc, in_maps, list(range(NCORES)))

    out = np.zeros((B, TD, V), np.float32)
    for c in range(NCORES):
        out[all_rows[c]] = res.results[c]["preds"]
    return out


# revision 15
# speedup vs baseline: 1.2212x; 1.2212x over previous
"""Trainium2 Bass kernel for an image-captioning LSTM decoder (v2).

Model (per reference):
  emb = embedding[captions]                      [B, T, E]
  sum_enc = encoder_out.sum(axis=1)              [B, ENC]
  h0 = mean_enc @ W_h0.T + b_h0 ; c0 likewise
  per step t (Tdec = T-1 steps):
    gates = [emb_t, sum_enc] @ W_ih.T + b_ih + h @ W_hh.T + b_hh
    i,f,g,o -> LSTM update; rows with t >= caption_len-1 give preds 0
    preds_t = h_new @ W_fc.T + b_fc  (masked)

Sharding: data-parallel over batch; core c owns rows {c, c+8, ...}.
Weights replicated; no inter-core communication.

v2 design (vs v1):
  * bf16 matmul operands everywhere (gates/fc/EG); fp32 psum accum and
    fp32 pointwise state.  Numpy-simulated rel err ~2.3e-3 (gate 2e-2).
  * "L layout" [32, 128] tiles for the pointwise chain: partition
    p = 8j + r holds (row r, feature-chunk j); DVE/ACT ops are ~2.5x
    cheaper than on [8, 512] tiles (measured 326 vs 805 ns).
  * Gate preacts computed straight into one [32, 512] PSUM tile
    (cols = [g|i|f|o]): first an identity-matmul injects EG_t (the
    precomputed emb/enc/bias part, fp32r), then 64 bf16 matmuls of
    128 cols accumulate h @ W_hh.T on top.  No DVE pre-adds; ACT reads
    PSUM directly.
  * One PE transpose per step ([32,128] -> [128,32]) instead of four,
    one cast into HTall (the bf16 hidden-state history that feeds both
    the next step's stationaries and the fc GEMM).
  * Host precomputes sum_enc / h0 / c0 / xenc (phase A eliminated);
    per-step EG_L tiles are gathered from pos-major EG via 4 small
    SBUF->SBUF DMAs, prefetched 2 steps ahead.
  * Phase B (EG GEMM) is split into 16 one-bank pieces; piece (0, *)
    runs up front, the rest fill the PE tail of steps 0..15 the same
    way fc chunks fill steps 16..62.
  * Masking moved out of the recurrence: inactive rows run free (their
    h is never read), preds are zeroed by folding the position mask
    into the fc PSUM->SBUF copy (tensor_scalar_mul / ACT-copy scale).
"""

import numpy as np
import ml_dtypes
from contextlib import ExitStack

import concourse.bass as bass
import concourse.tile as tile
from concourse import mybir, bacc
from concourse.bass_utils import run_bass_kernel_spmd
from concourse.masks import make_identity

F32 = mybir.dt.float32
F32R = mybir.dt.float32r
BF16 = mybir.dt.bfloat16
BF = ml_dtypes.bfloat16

NCORES = 8
B, T, V, E, D, ENC, P = 64, 64, 10000, 512, 512, 512, 196
TD = T - 1            # 63 decode steps
R = B // NCORES       # 8 rows per core
POS = TD * R          # 504 positions per core
KD = D // 128         # 4 k-chunks of the hidden dim
VCH = 500             # vocab chunk (psum bank holds 512 fp32)
NV = V // VCH         # 20 chunks
MTS = [128, 128, 128, POS - 384]   # fc position tiles (504 = 3*128 + 120)

_PROG_CACHE = {}


def _build_program(with_bfc: bool):
    nc = bacc.Bacc("TRN2", target_bir_lowering=False, debug=False,
                   num_devices=NCORES)

    def inp(name, shape, dt=F32):
        return nc.dram_tensor(name, shape, dt, kind="ExternalInput").ap()

    embT = inp("embT", [KD, 128, POS], BF16)
    wembT = inp("wembT", [KD, 128, 4 * D], BF16)
    whhT = inp("whhT", [KD, 128, 4 * D], BF16)
    wfcT = inp("wfcT", [KD, 128, V], BF16)
    xenc16 = inp("xenc16", [128, 4 * D], F32R)
    h0T = inp("h0T", [128, KD, R], BF16)
    c0L = inp("c0L", [128, 128], F32)
    sel32 = inp("sel32", [32, 128], F32R)
    maskP = inp("maskP", [128, 4], F32)
    if with_bfc:
        bfc = inp("bfc", [1, V], BF16)
        m1 = inp("m1", [1, POS], BF16)
    preds = nc.dram_tensor("preds", [R, TD, V], F32, kind="ExternalOutput").ap()
    preds_trv = preds.rearrange("r t v -> t r v")

    SIG = mybir.ActivationFunctionType.Sigmoid
    TANH = mybir.ActivationFunctionType.Tanh
    COPY = mybir.ActivationFunctionType.Copy

    with tile.TileContext(nc) as tc, ExitStack() as ctx:
        const_pool = ctx.enter_context(tc.tile_pool(name="const", bufs=1))
        state_pool = ctx.enter_context(tc.tile_pool(name="state", bufs=1))
        egl_pool = ctx.enter_context(tc.tile_pool(name="egl", bufs=5))
        pw_pool = ctx.enter_context(tc.tile_pool(name="pw", bufs=2))
        phd_out = ctx.enter_context(tc.tile_pool(name="phd_out", bufs=3))
        gps_pool = ctx.enter_context(tc.tile_pool(name="gps", bufs=2, space="PSUM"))
        tps_pool = ctx.enter_context(tc.tile_pool(name="tps", bufs=1, space="PSUM"))
        fps_pool = ctx.enter_context(tc.tile_pool(name="fps", bufs=3, space="PSUM"))
        bps_pool = ctx.enter_context(tc.tile_pool(name="bps", bufs=1, space="PSUM"))

        ENGS = [nc.sync, nc.gpsimd, nc.scalar, nc.sync]

        # ---- constants / state ----
        ident128 = const_pool.tile([128, 128], F32, name="ident128")
        make_identity(nc, ident128[:])
        identr128 = const_pool.tile([128, 128], F32R, name="identr128")
        nc.vector.tensor_copy(identr128[:], ident128[:])
        sel_sb = const_pool.tile([32, 128], F32R, name="sel_sb")
        nc.sync.dma_start(sel_sb[:], sel32[:])
        maskP_sb = const_pool.tile([128, 4], F32, name="maskP_sb")
        nc.sync.dma_start(maskP_sb[:], maskP[:])
        xenc_sb = const_pool.tile([128, 4 * D], F32R, name="xenc_sb")
        nc.gpsimd.dma_start(xenc_sb[:], xenc16[:])

        # HTall[:, k, 8s+r] = (h before step s).T chunk k, bf16 (k-major
        # so matmul stationary slices are 1-D contiguous)
        HTall = state_pool.tile([128, KD, T * R], BF16, name="HTall")
        nc.scalar.dma_start(HTall[:, :, 0:R], h0T[:])
        c_sb = state_pool.tile([128, 128], F32, name="c_sb")
        nc.gpsimd.dma_start(c_sb[:], c0L[:])

        whh_sb = [state_pool.tile([128, 4 * D], BF16, name=f"whh{k}")
                  for k in range(KD)]
        for k in range(KD):
            ENGS[k].dma_start(whh_sb[k][:], whhT[k])

        # resident bf16 W_fc: 80KB/partition (loaded after phase-B kickoff;
        # first needed at step 16)
        wfc_sb = [state_pool.tile([128, V], BF16, name=f"wfc{k}")
                  for k in range(KD)]

        # EG pos-major tiles: EG[m][8*s + r, 512j + 128n + c]
        EG = [state_pool.tile([128, 4 * D], F32R, name=f"EG{m}")
              for m in range(4)]
        # phase-B operands, loaded piecewise (m- / sl-granular)
        emb_sb = [state_pool.tile([128, POS], BF16, name=f"emb{k}")
                  for k in range(KD)]
        wemb_sb = [state_pool.tile([128, 4 * D], BF16, name=f"wemb{k}")
                   for k in range(KD)]

        def load_emb_piece(m):
            p0, pn = 128 * m, MTS[m]
            for k in range(KD):
                ENGS[k].dma_start(emb_sb[k][:, p0:p0 + pn],
                                  embT[k][:, p0:p0 + pn])

        def load_wemb_piece(sl):
            c0_, c1 = 512 * sl, 512 * (sl + 1)
            for k in range(KD):
                ENGS[(k + 2) % 4].dma_start(wemb_sb[k][:, c0_:c1],
                                            wembT[k][:, c0_:c1])

        def eg_piece_mm(m, sl):
            mw = MTS[m]
            ps = bps_pool.tile([128, 512], F32, name=f"egp{m}_{sl}", tag="egp")
            for k in range(KD):
                nc.tensor.matmul(ps[:mw, :],
                                 emb_sb[k][:, 128 * m:128 * m + mw],
                                 wemb_sb[k][:, 512 * sl:512 * (sl + 1)],
                                 start=(k == 0), stop=(k == KD - 1))
            return (ps, m, sl)

        def eg_piece_add(job):
            ps, m, sl = job
            mw = MTS[m]
            nc.vector.tensor_add(EG[m][:mw, 512 * sl:512 * (sl + 1)],
                                 ps[:mw, :], xenc_sb[:mw, 512 * sl:512 * (sl + 1)])

        def eg_piece(m, sl):
            eg_piece_add(eg_piece_mm(m, sl))

        if with_bfc:
            bfc_sb = const_pool.tile([1, V], BF16, name="bfc_sb")
            nc.sync.dma_start(bfc_sb[:], bfc[:])
            m1_sb = const_pool.tile([1, POS], BF16, name="m1_sb")
            nc.sync.dma_start(m1_sb[:], m1[:])

        def fc_mm(m, n):
            """fc matmuls for (pos-tile m, vocab chunk n); copy deferred."""
            mw = MTS[m]
            ps = fps_pool.tile([128, VCH], F32, name=f"fc{n}_{m}", tag="fcps")
            for k in range(KD):
                nc.tensor.matmul(
                    ps[:mw, :],
                    HTall[:, k, R * (16 * m + 1):R * (16 * m + 1) + mw],
                    wfc_sb[k][:, VCH * n:VCH * (n + 1)], start=(k == 0),
                    stop=(k == KD - 1 and not with_bfc))
            if with_bfc:
                nc.tensor.matmul(
                    ps[:mw, :], m1_sb[:, 128 * m:128 * m + mw],
                    bfc_sb[:, VCH * n:VCH * (n + 1)],
                    start=False, stop=True)
            return (ps, m, n)

        def fc_finish(job):
            """Masked PSUM->SBUF copy + output DMA (run after the casts)."""
            ps, m, n = job
            mw = MTS[m]
            stn = mw // R
            ot = phd_out.tile([128, VCH], F32, name=f"fo{n}_{m}", tag="fcout")
            if n % 2 == 0:
                nc.vector.tensor_scalar_mul(ot[:mw, :], ps[:mw, :],
                                            maskP_sb[:mw, m:m + 1])
            else:
                nc.scalar.activation(ot[:mw, :], ps[:mw, :], COPY,
                                     scale=maskP_sb[:mw, m:m + 1])
            t0 = 16 * m
            nc.gpsimd.dma_start(
                preds_trv[t0:t0 + stn, :, VCH * n:VCH * (n + 1)],
                ot[:mw, :])

        def egl_prefetch(t):
            """Gather EG_L[t] [32, 512] (partition 8j+r) from pos-major EG."""
            egl = egl_pool.tile([32, 512], F32R, name=f"egl{t}", tag="egl")
            m, s = t // 16, t % 16
            for j in range(KD):
                eng = nc.sync if j % 2 == 0 else nc.scalar
                eng.dma_start(egl[8 * j:8 * (j + 1), :],
                              EG[m][8 * s:8 * s + 8, 512 * j:512 * (j + 1)])
            return egl

        # ---- phase B piece (0, *) up front; EG_L prefetch for steps 0,1 ----
        load_emb_piece(0)
        load_wemb_piece(0)
        eg_piece(0, 0)
        load_wemb_piece(1)
        eg_piece(0, 1)
        load_wemb_piece(2)
        eg_piece(0, 2)
        load_wemb_piece(3)
        eg_piece(0, 3)
        egl_tiles = {0: egl_prefetch(0), 1: egl_prefetch(1), 2: egl_prefetch(2)}
        for m in range(1, 4):
            load_emb_piece(m)
        for blk in range(4):
            for k in range(KD):
                ENGS[(blk + k) % 3].dma_start(
                    wfc_sb[k][:, 2500 * blk:2500 * (blk + 1)],
                    wfcT[k][:, 2500 * blk:2500 * (blk + 1)])

        def g_inject(t):
            """Start step t's gate PSUM: G = sel32.T @ EG_L[t].

            G [128, 512]: partition 32j + r (rows r>=8 of each quadrant are
            zero garbage), col 128n + c."""
            G = gps_pool.tile([128, 512], F32, name=f"G{t}", tag="G")
            nc.tensor.matmul(G[:, :], sel_sb[:], egl_tiles[t][:],
                             start=True, stop=False, skip_group_check=True)
            return G

        # remaining phase-B pieces scheduled into steps 0..11 (one per step);
        # fc chunks into steps 16..62 (tile m during steps 16(m+1)..)
        PIECES = [(m, sl) for m in range(1, 4) for sl in range(4)]

        G_next = g_inject(0)
        for t in range(TD):
            G = G_next

            # gates: G[8j:8j+8, 128n:...] += hT_k.T @ Whh slice, bf16
            for k in range(KD):
                lhs = HTall[:, k, R * t:R * t + R]
                last = (k == KD - 1)
                for n in range(4):
                    if last and n == 3:
                        break
                    for j in range(KD):
                        nc.tensor.matmul(
                            G[32 * j:32 * j + 8, 128 * n:128 * (n + 1)],
                            lhs, whh_sb[k][:, 512 * j + 128 * n:
                                           512 * j + 128 * n + 128],
                            start=False, stop=last, skip_group_check=True,
                            tile_position=(0, 32 * j))

            # ACT on g/i/f while the o-bank matmuls + fillers stream
            TG = pw_pool.tile([128, 128], F32, name=f"tg{t}", tag="tg")
            nc.scalar.activation(TG[:], G[:, 0:128], TANH)
            SIF = pw_pool.tile([128, 256], F32, name=f"sif{t}", tag="sif")
            nc.scalar.activation(SIF[:], G[:, 128:384], SIG)

            # o-bank (k=3, n=3)
            lhs = HTall[:, KD - 1, R * t:R * t + R]
            for j in range(KD):
                nc.tensor.matmul(
                    G[32 * j:32 * j + 8, 384:512],
                    lhs, whh_sb[KD - 1][:, 512 * j + 384:512 * j + 512],
                    start=False, stop=True, skip_group_check=True,
                    tile_position=(0, 32 * j))

            # c path on DVE; tanh(c) queued on ACT before sigmoid(o) so the
            # h-mul's inputs are both ready right after the o matmuls land
            T1 = pw_pool.tile([128, 128], F32, name=f"t1_{t}", tag="t1")
            nc.vector.tensor_mul(T1[:], TG[:], SIF[:, 0:128])
            T2 = pw_pool.tile([128, 128], F32, name=f"t2_{t}", tag="t2")
            nc.vector.tensor_mul(T2[:], SIF[:, 128:256], c_sb[:])
            nc.vector.tensor_add(c_sb[:], T1[:], T2[:])
            TC = pw_pool.tile([128, 128], F32, name=f"tc{t}", tag="tc")
            nc.scalar.activation(TC[:], c_sb[:], TANH)
            SO = pw_pool.tile([128, 128], F32, name=f"so{t}", tag="so")
            nc.scalar.activation(SO[:], G[:, 384:512], SIG)

            # PE fillers: phase-B pieces (steps 2..13) / fc chunks (16..).
            # One filler + the next inject run before the transpose (covering
            # the sig_o -> h window); extra fc chunks go after it.  The
            # PSUM->SBUF copies are deferred past the HTall casts so the DVE
            # queue stays clear for the h chain.
            deferred = []
            fillers = []
            if 2 <= t < 2 + len(PIECES):
                fillers.append(("piece", PIECES[t - 2]))
            m = t // 16 - 1
            if m >= 0:
                L = min(16 * (m + 2), TD) - 16 * (m + 1)
                s = t - 16 * (m + 1)
                for n in range(s * NV // L, (s + 1) * NV // L):
                    fillers.append(("fc", (m, n)))
            if fillers:
                kind, arg = fillers[0]
                if kind == "fc":
                    deferred.append(("fc", fc_mm(*arg)))
                else:
                    deferred.append(("piece", eg_piece_mm(*arg)))
            # next step's EG inject can run before this step's transpose
            if t + 1 < TD:
                G_next = g_inject(t + 1)

            HB = pw_pool.tile([128, 128], F32R, name=f"h{t}", tag="hb")
            nc.vector.tensor_mul(HB[:], TC[:], SO[:])

            tp = tps_pool.tile([128, 128], F32R, name=f"hT{t}", tag="htp")
            nc.tensor.transpose(tp[:, :], HB[:], identr128[:])
            for k in range(KD):
                nc.vector.tensor_copy(
                    HTall[:, k, R * (t + 1):R * (t + 2)],
                    tp[:, 32 * k:32 * k + R])

            for kind, arg in fillers[1:]:
                if kind == "fc":
                    deferred.append(("fc", fc_mm(*arg)))
                else:
                    deferred.append(("piece", eg_piece_mm(*arg)))
            for kind, job in deferred:
                if kind == "fc":
                    fc_finish(job)
                else:
                    eg_piece_add(job)

            if t + 3 < TD:
                egl_tiles[t + 3] = egl_prefetch(t + 3)
            egl_tiles.pop(t, None)

        for n in range(NV):
            fc_finish(fc_mm(3, n))

    nc.compile()
    return nc


GPERM = None  # row permutation [g, i, f, o] built lazily


def _gate_perm():
    global GPERM
    if GPERM is None:
        gp = np.concatenate([np.arange(2 * D, 3 * D), np.arange(0, D),
                             np.arange(D, 2 * D), np.arange(3 * D, 4 * D)])
        # [n, j, c] -> [j, n, c]
        GPERM = gp.reshape(4, 4, 128).transpose(1, 0, 2).reshape(4 * D)
    return GPERM


def _chunkT(w, dt=BF):
    """[N, K<=512] weight -> transposed chunks [KD, 128, N]."""
    wt = np.ascontiguousarray(w.T.astype(np.float32))
    return wt.reshape(KD, 128, w.shape[0]).astype(dt)


def kernel(encoder_out, encoder_captions, caption_len, embedding,
           W_ih, b_ih, W_hh, b_hh, W_h0, b_h0, W_c0, b_c0, W_fc, b_fc):
    encoder_out = np.asarray(encoder_out, dtype=np.float32)
    encoder_captions = np.asarray(encoder_captions)
    caption_len = np.asarray(caption_len)
    embedding = np.asarray(embedding, dtype=np.float32)
    W_ih = np.asarray(W_ih, dtype=np.float32); b_ih = np.asarray(b_ih, np.float32)
    W_hh = np.asarray(W_hh, dtype=np.float32); b_hh = np.asarray(b_hh, np.float32)
    W_h0 = np.asarray(W_h0, dtype=np.float32); b_h0 = np.asarray(b_h0, np.float32)
    W_c0 = np.asarray(W_c0, dtype=np.float32); b_c0 = np.asarray(b_c0, np.float32)
    W_fc = np.asarray(W_fc, dtype=np.float32); b_fc = np.asarray(b_fc, np.float32)

    with_bfc = bool(np.any(b_fc != 0))
    key = with_bfc
    if key not in _PROG_CACHE:
        _PROG_CACHE[key] = _build_program(with_bfc)
    nc = _PROG_CACHE[key]

    perm = _gate_perm()
    whhT = _chunkT(W_hh[perm])
    wembT = _chunkT(W_ih[perm, :E])
    wfcT = _chunkT(W_fc)

    # host phase A
    sum_enc = encoder_out.sum(axis=1)                       # [B, ENC]
    mean_enc = sum_enc / np.float32(P)
    h0 = mean_enc @ W_h0.T + b_h0                            # [B, D]
    c0 = mean_enc @ W_c0.T + b_c0
    xenc = (sum_enc @ W_ih[:, E:].T + b_ih + b_hh)[:, perm]  # [B, 4D]

    in_maps = []
    all_rows = []
    for c in range(NCORES):
        rows = np.arange(c, B, NCORES)
        all_rows.append(rows)
        cap = np.asarray(encoder_captions[rows][:, :TD], dtype=np.int64)
        embg = embedding[cap]                                # [R, TD, E]
        embT = np.ascontiguousarray(
            embg.transpose(2, 1, 0).reshape(E, POS)).reshape(
                KD, 128, POS).astype(BF)
        dec_len = (caption_len[rows] - 1).astype(np.int64)
        mp = np.zeros((64, R), np.float32)
        mp[:TD] = (np.arange(TD)[:, None] < dec_len[None, :])
        maskP = np.ascontiguousarray(
            mp.reshape(4, 16, R).transpose(1, 2, 0).reshape(128, 4))
        h0c = h0[rows]                                       # [R, D]
        h0T = np.ascontiguousarray(
            h0c.reshape(R, KD, 128).transpose(2, 1, 0)).astype(BF)  # [128,KD,R]
        c0c = c0[rows]
        c0L = np.zeros((KD, 32, 128), np.float32)
        c0L[:, :R, :] = c0c.reshape(R, KD, 128).transpose(1, 0, 2)
        c0L = np.ascontiguousarray(c0L.reshape(128, 128))
        xenc16 = np.ascontiguousarray(np.tile(xenc[rows], (16, 1)))  # [128,4D]
        sel32 = np.zeros((32, 128), np.float32)
        for j in range(KD):
            for r in range(R):
                sel32[8 * j + r, 32 * j + r] = 1.0
        im = dict(embT=embT, wembT=wembT, whhT=whhT, wfcT=wfcT,
                  xenc16=xenc16, h0T=h0T, c0L=c0L, maskP=maskP, sel32=sel32)
        if with_bfc:
            im["bfc"] = b_fc.reshape(1, V).astype(BF)
            im["m1"] = np.ones((1, POS), BF)
        in_maps.append(im)

    global _LAST_IN_MAPS
    _LAST_IN_MAPS = in_maps
    res = run_bass_kernel_spmd(nc, in_maps, list(range(NCORES)))

    out = np.zeros((B, TD, V), np.float32)
    for c in range(NCORES):
        out[all_rows[c]] = res.results[c]["preds"]
    return out
